# revision 14
# baseline (speedup 1.0000x reference)
"""Trainium2 Bass kernel for the pointer-network decoder (nn_Decoder).

Problem: teacher-forced pointer-network decode.
  T=65 sequential LSTM steps (D=2048, 4D=8192 gate rows), each step followed
  by additive attention over L+1=513 encoder positions with V=2048 hidden.

Parallelization (8 NeuronCores, tensor-parallel):
  - LSTM: gate dimension sharded 8-way (1024 gate rows/core = 256 dims of
    each of i,f,g,o). W_hh shard lives in SBUF as PE lhsT tiles; per-step
    matvec is 128 weight-stationary matmuls accumulating into PSUM [128, 8].
    After the elementwise update each core owns h[ c*256 : (c+1)*256 ]; a
    per-step AllGather rebuilds the full h on every core.
  - Pointer: V sharded 8-way (256 cols/core). enc_proj = We_w @ enc^T is
    computed once ([256v, 513l] layout, v on partitions); per (t, v-tile)
    one scalar-engine Tanh with per-partition bias d[v, t] fuses the
    "enc + Wd h" add; the weighted v-reduction is a PE matmul against v_w.
    Partial logits [65, 513] are AllReduce-summed across cores.
  - Losses (logsumexp - target logit) computed on device from full logits.

The host side only reshapes/shards inputs (incl. the teacher-forced input
projection gate_x = xs @ W_ih^T + b, a [65,3]x[3,8192] matmul) and takes
core 0's outputs.
"""

import os
import sys
import types

import numpy as np

# ---------------------------------------------------------------------------
# Compatibility patches for this container's walrus build (accepts at most one
# sem-ge wait per instruction; rejects sem-eq-imm). The bass repo's all-engine
# barrier emits Drains with an eq-imm wait, and TileContext's tail drain can
# carry many waits. Also installs the NTFF profiling hook shim so trace=True
# works under axon (the agent image's antenv lacks axon_hooks).
# ---------------------------------------------------------------------------

import bass_rust  # noqa: E402
import concourse.bass as bass  # noqa: E402
import concourse.mybir as mybir  # noqa: E402
import concourse.tile as tile  # noqa: E402
from concourse.vector_clock import ScopedClock  # noqa: E402
from concourse.bass_utils import run_bass_kernel_spmd  # noqa: E402


def _install_ntff_hook_shim():
    try:
        import antenv.axon_hooks  # noqa: F401
        return
    except ImportError:
        pass
    mod = types.ModuleType("antenv.axon_hooks")
    _hook = [None]
    mod.set_axon_ntff_profile_hook = lambda h: _hook.__setitem__(0, h)
    mod.get_axon_ntff_profile_hook = lambda: _hook[0]
    sys.modules["antenv.axon_hooks"] = mod
    import antenv

    antenv.axon_hooks = mod
    try:
        from trn_agent_boot.trn_boot import _ntff_profile_via_ctypes

        h = _ntff_profile_via_ctypes("/opt/axon/libaxon_pjrt.so")
        if h is not None:
            mod.set_axon_ntff_profile_hook(h)
    except Exception:
        pass


def _compat_multi_engine_barrier(self, engines):
    bar = getattr(self, "_compat_bar", None)
    if bar is None:
        bar = self._compat_bar = [self.alloc_semaphore("compat_barrier_sem"), 0]
    sem = bar[0]
    target = bar[1] + len(engines)
    for e in engines:
        self.engines[e].drain().then_inc(sem, 1)
    for e in engines:
        self.engines[e].wait_ge(sem, target)
    bar[1] = target


def _compat_all_engine_barrier(self, *, sem_only: bool = False):
    _compat_multi_engine_barrier(self, list(self.engines))


def _compat_dummy_sem(self, nc):
    sem = getattr(nc, "_compat_dummy_sem_handle", None)
    if sem is None:
        sem = nc._compat_dummy_sem_handle = nc.alloc_semaphore("compat_dummy_sem")
    return sem


def _compat_drain_and_barrier(self, tick_clock, wait_clock):
    nc = self.nc
    nc.sync.drain()
    carrier = nc.sync.wait_ge(self._compat_dummy_sem(nc), 0)
    wait_clock.add_sem_waits(carrier.ins, ScopedClock({None: tick_clock.global_clock}))
    si = carrier.ins.sync_info
    waits = [w for w in si.on_wait if w.ant_name != "compat_dummy_sem"]
    if waits:
        si.on_wait = [waits[0]]
        for w in waits[1:]:
            extra = nc.sync.wait_ge(self._compat_dummy_sem(nc), 0)
            extra.ins.sync_info.on_wait = [w]
    else:
        si.on_wait = []

    nc.all_engine_barrier()
    assert self.sems is not None
    popped = nc._tile_sem_poison_stack.pop()
    assert popped is self._sem_poison
    nc.clear_and_free_semaphores(list(self.sems.allocated().values()))
    nc.all_engine_barrier()


def _split_multi_waits(nc):
    """This walrus build supports at most one sync wait per instruction.
    Tile's wait-assignment can attach several; split the extras into
    standalone EventSemaphore instructions on the same engine, inserted
    immediately before (engines execute their stream in order)."""
    n = [0]
    for f in nc.m.functions:
        for bb in f.blocks:
            out = []
            changed = False
            for inst in bb.instructions:
                si = inst.sync_info
                if si is not None and len(si.on_wait) > 1:
                    waits = list(si.on_wait)
                    for w in waits[:-1]:
                        n[0] += 1
                        ev = mybir.InstEventSemaphore(
                            name=f"EVSPLIT-{n[0]}", ins=[], outs=[]
                        )
                        ev.engine = inst.engine
                        ev.sync_info = bass_rust.SyncInfo(on_wait=[w], on_update=[])
                        nc.register_instruction(ev, overwrite=True)
                        out.append(ev)
                    si.on_wait = [waits[-1]]
                    changed = True
                out.append(inst)
            if changed:
                bb.instructions = out


_install_ntff_hook_shim()
bass.Bass.multi_engine_barrier = _compat_multi_engine_barrier
bass.Bass.all_engine_barrier = _compat_all_engine_barrier
tile.TileContext._drain_and_barrier = _compat_drain_and_barrier
tile.TileContext._compat_dummy_sem = _compat_dummy_sem

# ---------------------------------------------------------------------------
# Problem constants (hardcoded per the grading contract).
# ---------------------------------------------------------------------------

D = 2048          # hidden_d
V = 2048          # hidden_v
L = 512           # seq length
LP1 = L + 1       # 513
T = 65            # decode steps (len(positions) + 1)
NC = 8            # cores
GS = 4 * D // NC  # 1024 gate rows per core
DS = D // NC      # 256 h dims per core
VS = V // NC      # 256 v dims per core
KT = D // 128     # 16 contraction tiles
MT = GS // 128    # 8 gate tiles per core
VT = VS // 128    # 2 v tiles per core

FP32 = mybir.dt.float32
BF16 = mybir.dt.bfloat16
AF = mybir.ActivationFunctionType
ALU = mybir.AluOpType

_cache = {}


def _build_program():
    nc = bass.Bass(num_devices=NC)

    # ---- I/O ----
    inp = {}
    def din(name, shape):
        inp[name] = nc.dram_tensor(name, list(shape), FP32, kind="ExternalInput")
        return inp[name]

    w_sb_in = nc.dram_tensor("w_sb", [128, KT * MT * 128], BF16,
                             kind="ExternalInput")  # W_hh lhsT tiles (kt, m)
    gx_in = din("gx", [128, MT, T])                  # gate_x bias per (p, m, t)
    h0_in = din("h0", [128, KT])                     # initial h chunks
    wet_in = din("wet", [128, KT * VS])              # We_w shard lhsT tiles
    enct_in = din("enct", [128, KT * LP1])           # encoder_states^T tiles
    wdt_in = din("wdt", [128, KT * VS])              # Wd_w shard lhsT tiles
    vw_in = din("vw", [128, VT])                     # v_w shard
    biasd_in = din("biasd", [128, VT])               # We_b + Wd_b shard
    vb_in = din("vb", [T, 1])                        # v_b broadcast
    tmask_in = din("tmask", [T, LP1])                # one-hot targets
    ident_in = din("ident", [128, 128])              # identity for PE transpose

    y_logits = nc.dram_tensor("y_logits", [T, LP1], FP32, kind="ExternalOutput")
    y_hs = nc.dram_tensor("y_hs", [T, D], FP32, kind="ExternalOutput")
    y_losses = nc.dram_tensor("y_losses", [T, 1], FP32, kind="ExternalOutput")

    rg = [list(range(NC))]

    with tile.TileContext(nc) as tc:
        with (
            tc.tile_pool(name="const", bufs=1) as cpool,
            tc.tile_pool(name="work", bufs=2) as wpool,
            tc.tile_pool(name="tanhp", bufs=3) as tpool,
            tc.tile_pool(name="ps_small", bufs=2, space="PSUM") as ps_small,
            tc.tile_pool(name="ps_enc", bufs=1, space="PSUM") as ps_enc,
            tc.tile_pool(name="ps_lg", bufs=2, space="PSUM") as ps_lg,
            tc.tile_pool(name="dram", bufs=1, space="DRAM") as dram,
        ):
            # ---- resident SBUF tensors ----
            w_sb = cpool.tile([128, KT * MT * 128], BF16)
            gx_sb = cpool.tile([128, MT, T], FP32)
            wet_sb = cpool.tile([128, KT * VS], FP32)
            enct_sb = cpool.tile([128, KT * LP1], FP32)
            wdt_sb = cpool.tile([128, KT * VS], FP32)
            vw_sb = cpool.tile([128, VT], FP32)
            biasd_sb = cpool.tile([128, VT], FP32)
            vb_sb = cpool.tile([T, 1], FP32)
            tmask_sb = cpool.tile([T, LP1], FP32)
            ident_sb = cpool.tile([128, 128], FP32)
            enc_sb = cpool.tile([128, VT, LP1], FP32)   # enc_proj [v, l]
            ht_sb = cpool.tile([128, KT, T], FP32)      # H^T chunks over time
            d_sb = cpool.tile([128, VT, T], FP32)       # Wd h + bias per (v, t)
            c_sb = cpool.tile([128, 2], FP32)           # cell state shard
            h0_sb = cpool.tile([128, KT], FP32)

            nc.sync.dma_start(w_sb[:], w_sb_in[:])
            nc.sync.dma_start(gx_sb[:], gx_in[:])
            nc.sync.dma_start(wet_sb[:], wet_in[:])
            nc.sync.dma_start(enct_sb[:], enct_in[:])
            nc.sync.dma_start(wdt_sb[:], wdt_in[:])
            nc.sync.dma_start(vw_sb[:], vw_in[:])
            nc.sync.dma_start(biasd_sb[:], biasd_in[:])
            nc.sync.dma_start(vb_sb[:], vb_in[:])
            nc.sync.dma_start(tmask_sb[:], tmask_in[:])
            nc.sync.dma_start(ident_sb[:], ident_in[:])
            nc.sync.dma_start(h0_sb[:], h0_in[:])
            nc.gpsimd.memset(c_sb[:], 0.0)

            # ---- encoder projection: enc_sb[v, l] = (We_w @ enc^T) shard ----
            # (matmul PSUM outputs must stay within one 2KB bank -> chunk l)
            NCHUNKS = ((0, 257), (257, LP1))
            for vt in range(VT):
                for n0, n1 in NCHUNKS:
                    enc_ps = ps_enc.tile([128, 257], FP32, name="enc_ps")
                    for kt in range(KT):
                        nc.tensor.matmul(
                            enc_ps[:, 0 : n1 - n0],
                            wet_sb[:, kt * VS + vt * 128 : kt * VS + vt * 128 + 128],
                            enct_sb[:, kt * LP1 + n0 : kt * LP1 + n1],
                            start=(kt == 0),
                            stop=(kt == KT - 1),
                        )
                    nc.vector.tensor_copy(enc_sb[:, vt, n0:n1], enc_ps[:, 0 : n1 - n0])

            # ---- sequential LSTM with fused per-step pointer work ----
            # Exchange bf16 (hi, lo) h pairs so the gathered data feeds the
            # PE matvec directly; reconstruct fp32 h as hi+lo. Each step's
            # pointer work (d column, 2 tanh tiles, v_w reduction) runs in
            # the exchange/latency shadow on otherwise-idle engines.
            lar_in = dram.tile([T, LP1], FP32, name="lar_in", tag="lar_in")
            lar_out = dram.tile([T, LP1], FP32, name="lar_out", tag="lar_out",
                                addr_space="Shared")

            # t=0 rhs from the initial state
            rhs_cur = wpool.tile([128, KT, 2], BF16, name="rhs_hl", bufs=3)
            nc.vector.tensor_copy(rhs_cur[:, :, 0], h0_sb[:])
            herr0 = wpool.tile([128, KT], FP32, name="herr0")
            nc.vector.tensor_tensor(herr0[:], h0_sb[:], rhs_cur[:, :, 0], ALU.subtract)
            nc.vector.tensor_copy(rhs_cur[:, :, 1], herr0[:])

            for t in range(T):
                # gates matvec: psum[:, 2m:2m+2] = sum_kt W(kt,m)^T @ [h_hi h_lo]
                g_ps = ps_small.tile([128, 2 * MT], FP32, name="g_ps", tag="mm1")
                for m in range(MT):
                    for kt in range(KT):
                        nc.tensor.matmul(
                            g_ps[:, 2 * m : 2 * m + 2],
                            w_sb[:, (kt * MT + m) * 128 : (kt * MT + m) * 128 + 128],
                            rhs_cur[:, kt, :],
                            start=(kt == 0),
                            stop=(kt == KT - 1),
                        )
                gates0 = wpool.tile([128, MT], FP32, name="gates0")
                nc.vector.tensor_reduce(
                    gates0[:], g_ps[:].rearrange("p (m two) -> p m two", two=2),
                    mybir.AxisListType.X, ALU.add,
                )
                gates = wpool.tile([128, MT], FP32, name="gates")
                nc.vector.tensor_tensor(gates[:], gates0[:], gx_sb[:, :, t], ALU.add)
                acts = wpool.tile([128, MT], FP32, name="acts")
                nc.scalar.activation(acts[:, 0:4], gates[:, 0:4], AF.Sigmoid)
                nc.scalar.activation(acts[:, 4:6], gates[:, 4:6], AF.Tanh)
                nc.scalar.activation(acts[:, 6:8], gates[:, 6:8], AF.Sigmoid)
                # c = f*c + i*g ; h = o * tanh(c)
                tmp = wpool.tile([128, 4], FP32, name="tmp")
                nc.vector.tensor_tensor(tmp[:, 0:2], acts[:, 2:4], c_sb[:], ALU.mult)
                nc.vector.tensor_tensor(tmp[:, 2:4], acts[:, 0:2], acts[:, 4:6], ALU.mult)
                nc.vector.tensor_tensor(c_sb[:], tmp[:, 0:2], tmp[:, 2:4], ALU.add)
                tanh_c = wpool.tile([128, 2], FP32, name="tanh_c")
                nc.scalar.activation(tanh_c[:], c_sb[:], AF.Tanh)
                h_sh = wpool.tile([128, 2], FP32, name="h_sh")
                nc.vector.tensor_tensor(h_sh[:], acts[:, 6:8], tanh_c[:], ALU.mult)

                # bf16 hi/lo split of the own shard, then AllGather
                sh_hl = wpool.tile([128, 2, 2], BF16, name="sh_hl", bufs=3)
                nc.vector.tensor_copy(sh_hl[:, :, 0], h_sh[:])
                sherr = wpool.tile([128, 2], FP32, name="sherr")
                nc.vector.tensor_tensor(sherr[:], h_sh[:], sh_hl[:, :, 0], ALU.subtract)
                nc.vector.tensor_copy(sh_hl[:, :, 1], sherr[:])
                ag_in = dram.tile([128, 2, 2], BF16, name=f"agi{t}", tag=f"agi{t}")
                ag_out = dram.tile([NC, 128, 2, 2], BF16, name=f"ago{t}",
                                   tag=f"ago{t}", addr_space="Shared")
                nc.sync.dma_start(ag_in[:], sh_hl[:])
                nc.gpsimd.collective_compute(
                    "AllGather", ALU.bypass, replica_groups=rg,
                    ins=[ag_in[:].opt()], outs=[ag_out[:].opt()],
                )
                rhs_cur = wpool.tile([128, KT, 2], BF16, name="rhs_hl", bufs=3)
                nc.sync.dma_start(
                    rhs_cur[:].rearrange("p (c j) l -> p c j l", j=2),
                    ag_out[:].rearrange("c p j l -> p c j l"),
                )
                # fp32 h for this step = hi + lo (for hs output and d matvec)
                nc.vector.tensor_tensor(ht_sb[:, :, t], rhs_cur[:, :, 0],
                                        rhs_cur[:, :, 1], ALU.add)

                # pointer work for step t (runs in the next step's shadow)
                lg_psA = ps_lg.tile([1, 257], FP32, name="lg_psA", bufs=1)
                lg_psB = ps_lg.tile([1, 256], FP32, name="lg_psB", bufs=1)
                for vt in range(VT):
                    d_ps = ps_small.tile([128, 1], FP32, name="d_ps", tag="dcol",
                                         bufs=1)
                    for kt in range(KT):
                        nc.tensor.matmul(
                            d_ps[:],
                            wdt_sb[:, kt * VS + vt * 128 : kt * VS + vt * 128 + 128],
                            ht_sb[:, kt, t : t + 1],
                            start=(kt == 0),
                            stop=(kt == KT - 1),
                        )
                    nc.vector.tensor_scalar(
                        d_sb[:, vt, t : t + 1], d_ps[:],
                        biasd_sb[:, vt : vt + 1], None, ALU.add,
                    )
                    tanh_t = tpool.tile([128, LP1], FP32, name="tanh_t")
                    nc.scalar.activation(
                        tanh_t[:], enc_sb[:, vt, :], AF.Tanh,
                        bias=d_sb[:, vt, t : t + 1],
                    )
                    for (n0, n1), lg_ps in ((NCHUNKS[0], lg_psA), (NCHUNKS[1], lg_psB)):
                        nc.tensor.matmul(
                            lg_ps[0:1, 0 : n1 - n0],
                            vw_sb[:, vt : vt + 1],
                            tanh_t[:, n0:n1],
                            start=(vt == 0),
                            stop=(vt == VT - 1),
                        )
                lrow = tpool.tile([1, LP1], FP32, name="lrow")
                nc.vector.tensor_copy(lrow[0:1, 0:257], lg_psA[0:1, :])
                nc.vector.tensor_copy(lrow[0:1, 257:LP1], lg_psB[0:1, :])
                nc.sync.dma_start(lar_in[t : t + 1, :], lrow[0:1, :])

            # ---- AllReduce partial logits ----
            nc.gpsimd.collective_compute(
                "AllReduce", ALU.add, replica_groups=rg,
                ins=[lar_in[:].opt()], outs=[lar_out[:].opt()],
            )
            lraw_sb = cpool.tile([T, LP1], FP32)
            nc.sync.dma_start(lraw_sb[:], lar_out[:])
            # + v_b
            lfull_sb = cpool.tile([T, LP1], FP32)
            nc.scalar.activation(lfull_sb[:], lraw_sb[:], AF.Identity, bias=vb_sb[:])
            nc.sync.dma_start(y_logits[:], lfull_sb[:])

            # ---- losses: logsumexp(logits) - logits[target] ----
            mx = wpool.tile([T, 1], FP32, name="mx")
            nc.vector.tensor_reduce(mx[:], lfull_sb[:], mybir.AxisListType.X, ALU.max)
            negm = wpool.tile([T, 1], FP32, name="negm")
            nc.vector.tensor_scalar_mul(negm[:], mx[:], -1.0)
            es = wpool.tile([T, LP1], FP32, name="es")
            sumexp = wpool.tile([T, 1], FP32, name="sumexp")
            nc.scalar.activation(es[:], lfull_sb[:], AF.Exp, bias=negm[:],
                                 accum_out=sumexp[:])
            lse = wpool.tile([T, 1], FP32, name="lse")
            nc.scalar.activation(lse[:], sumexp[:], AF.Ln)
            tl_prod = wpool.tile([T, LP1], FP32, name="tl_prod")
            nc.vector.tensor_tensor(tl_prod[:], lfull_sb[:], tmask_sb[:], ALU.mult)
            tl = wpool.tile([T, 1], FP32, name="tl")
            nc.vector.tensor_reduce(tl[:], tl_prod[:], mybir.AxisListType.X, ALU.add)
            lsum = wpool.tile([T, 1], FP32, name="lsum")
            nc.vector.tensor_tensor(lsum[:], lse[:], mx[:], ALU.add)
            loss = wpool.tile([T, 1], FP32, name="loss")
            nc.vector.tensor_tensor(loss[:], lsum[:], tl[:], ALU.subtract)
            nc.sync.dma_start(y_losses[:], loss[:])

            # ---- hs output: transpose H^T chunks -> [65, 2048] ----
            for kt in range(KT):
                tr_ps = ps_small.tile([T, 128], FP32, name="tr_ps", tag="mm1")
                nc.tensor.transpose(tr_ps[:], ht_sb[:, kt, :], ident_sb[:])
                hs_sb = wpool.tile([T, 128], FP32, name="hs_sb")
                nc.vector.tensor_copy(hs_sb[:], tr_ps[:])
                nc.sync.dma_start(y_hs[:, kt * 128 : (kt + 1) * 128], hs_sb[:])

    _split_multi_waits(nc)
    return nc


def _prep_inputs_core(c, arrs):
    """Build the per-core input map from full numpy inputs."""
    (initial_state, encoder_states, seq_points, positions,
     W_ih, W_hh, b_ih, b_hh, We_w, We_b, Wd_w, Wd_b, v_w, v_b) = arrs

    special = np.array([0.0, 0.0, 1.0], dtype=np.float32)
    stack_seq = np.concatenate([seq_points, special[None]], axis=0)
    xs = np.concatenate([special[None], stack_seq[positions]], axis=0)  # [T, 3]
    targets = np.concatenate([positions.astype(np.int64), [L]], axis=0)

    gate_x = xs @ W_ih.T + b_ih + b_hh                     # [T, 4D]

    # gate-row indices for core c: 256-chunk c of each of i,f,g,o
    idx = np.concatenate([
        np.arange(g * D + c * DS, g * D + (c + 1) * DS) for g in range(4)
    ])
    W_shard = W_hh[idx, :]                                  # [1024, 2048]
    w_sb = (W_shard.reshape(MT, 128, KT, 128)
            .transpose(3, 2, 0, 1).reshape(128, KT * MT * 128))
    gx = gate_x[:, idx].reshape(T, MT, 128).transpose(2, 1, 0)  # [128, MT, T]
    h0 = initial_state.reshape(KT, 128).T                   # [128, KT]

    We_shard = We_w[c * VS:(c + 1) * VS, :]
    wet = (We_shard.T.reshape(KT, 128, VS)
           .transpose(1, 0, 2).reshape(128, KT * VS))
    enct = (encoder_states.T.reshape(KT, 128, LP1)
            .transpose(1, 0, 2).reshape(128, KT * LP1))
    Wd_shard = Wd_w[c * VS:(c + 1) * VS, :]
    wdt = (Wd_shard.T.reshape(KT, 128, VS)
           .transpose(1, 0, 2).reshape(128, KT * VS))
    vw = v_w[0, c * VS:(c + 1) * VS].reshape(VT, 128).T     # [128, VT]
    biasd = (We_b[c * VS:(c + 1) * VS] + Wd_b[c * VS:(c + 1) * VS]
             ).reshape(VT, 128).T
    vb = np.full((T, 1), np.float32(v_b[0]))
    tmask = np.zeros((T, LP1), dtype=np.float32)
    tmask[np.arange(T), targets] = 1.0

    import ml_dtypes
    f32c = lambda a: np.ascontiguousarray(a, dtype=np.float32)
    return {
        "w_sb": np.ascontiguousarray(w_sb.astype(ml_dtypes.bfloat16)), "gx": f32c(gx), "h0": f32c(h0),
        "wet": f32c(wet), "enct": f32c(enct), "wdt": f32c(wdt),
        "vw": f32c(vw), "biasd": f32c(biasd), "vb": f32c(vb),
        "tmask": f32c(tmask), "ident": np.eye(128, dtype=np.float32),
    }


LAST_EXEC_NS = None


def kernel(initial_state, encoder_states, seq_points, positions,
           W_ih, W_hh, b_ih, b_hh, We_w, We_b, Wd_w, Wd_b, v_w, v_b):
    global LAST_EXEC_NS
    arrs = [np.asarray(a) for a in (
        initial_state, encoder_states, seq_points, positions,
        W_ih, W_hh, b_ih, b_hh, We_w, We_b, Wd_w, Wd_b, v_w, v_b)]
    arrs = [a.astype(np.float32) if a.dtype == np.float64 else a for a in arrs]
    positions_np = np.asarray(arrs[3]).astype(np.int64)
    arrs[3] = positions_np

    if "nc" not in _cache:
        _cache["nc"] = _build_program()
    nc = _cache["nc"]

    in_maps = [_prep_inputs_core(c, arrs) for c in range(NC)]
    trace = bool(int(os.environ.get("KERNEL_TRACE", "0")))
    res = run_bass_kernel_spmd(nc, in_maps, core_ids=list(range(NC)), trace=trace)
    LAST_EXEC_NS = res.exec_time_ns
    out = res.results[0]
    logits = out["y_logits"].astype(np.float32)
    hs = out["y_hs"].astype(np.float32)
    losses = out["y_losses"].reshape(T).astype(np.float32)
    return logits, hs, losses


# revision 16
# speedup vs baseline: 1.4825x; 1.4825x over previous
"""Trainium2 Bass kernel for the pointer-network decoder (nn_Decoder).

Problem: teacher-forced pointer-network decode.
  T=65 sequential LSTM steps (D=2048, 4D=8192 gate rows), each step followed
  by additive attention over L+1=513 encoder positions with V=2048 hidden.

Parallelization (8 NeuronCores, tensor-parallel):
  - LSTM: gate dimension sharded 8-way (1024 gate rows/core = 256 dims of
    each of i,f,g,o). W_hh shard lives in SBUF as PE lhsT tiles; per-step
    matvec is 128 weight-stationary matmuls accumulating into PSUM [128, 8].
    After the elementwise update each core owns h[ c*256 : (c+1)*256 ]; a
    per-step AllGather rebuilds the full h on every core.
  - Pointer: V sharded 8-way (256 cols/core). enc_proj = We_w @ enc^T is
    computed once ([256v, 513l] layout, v on partitions); per (t, v-tile)
    one scalar-engine Tanh with per-partition bias d[v, t] fuses the
    "enc + Wd h" add; the weighted v-reduction is a PE matmul against v_w.
    Partial logits [65, 513] are AllReduce-summed across cores.
  - Losses (logsumexp - target logit) computed on device from full logits.

The host side only reshapes/shards inputs (incl. the teacher-forced input
projection gate_x = xs @ W_ih^T + b, a [65,3]x[3,8192] matmul) and takes
core 0's outputs.
"""

import os
import sys
import types

import numpy as np

# ---------------------------------------------------------------------------
# Compatibility patches for this container's walrus build (accepts at most one
# sem-ge wait per instruction; rejects sem-eq-imm). The bass repo's all-engine
# barrier emits Drains with an eq-imm wait, and TileContext's tail drain can
# carry many waits. Also installs the NTFF profiling hook shim so trace=True
# works under axon (the agent image's antenv lacks axon_hooks).
# ---------------------------------------------------------------------------

import bass_rust  # noqa: E402
import concourse.bass as bass  # noqa: E402
import concourse.mybir as mybir  # noqa: E402
import concourse.tile as tile  # noqa: E402
from concourse.vector_clock import ScopedClock  # noqa: E402
from concourse.bass_utils import run_bass_kernel_spmd  # noqa: E402


def _install_ntff_hook_shim():
    try:
        import antenv.axon_hooks  # noqa: F401
        return
    except ImportError:
        pass
    mod = types.ModuleType("antenv.axon_hooks")
    _hook = [None]
    mod.set_axon_ntff_profile_hook = lambda h: _hook.__setitem__(0, h)
    mod.get_axon_ntff_profile_hook = lambda: _hook[0]
    sys.modules["antenv.axon_hooks"] = mod
    import antenv

    antenv.axon_hooks = mod
    try:
        from trn_agent_boot.trn_boot import _ntff_profile_via_ctypes

        h = _ntff_profile_via_ctypes("/opt/axon/libaxon_pjrt.so")
        if h is not None:
            mod.set_axon_ntff_profile_hook(h)
    except Exception:
        pass


def _compat_multi_engine_barrier(self, engines):
    bar = getattr(self, "_compat_bar", None)
    if bar is None:
        bar = self._compat_bar = [self.alloc_semaphore("compat_barrier_sem"), 0]
    sem = bar[0]
    target = bar[1] + len(engines)
    for e in engines:
        self.engines[e].drain().then_inc(sem, 1)
    for e in engines:
        self.engines[e].wait_ge(sem, target)
    bar[1] = target


def _compat_all_engine_barrier(self, *, sem_only: bool = False):
    _compat_multi_engine_barrier(self, list(self.engines))


def _compat_dummy_sem(self, nc):
    sem = getattr(nc, "_compat_dummy_sem_handle", None)
    if sem is None:
        sem = nc._compat_dummy_sem_handle = nc.alloc_semaphore("compat_dummy_sem")
    return sem


def _compat_drain_and_barrier(self, tick_clock, wait_clock):
    nc = self.nc
    nc.sync.drain()
    carrier = nc.sync.wait_ge(self._compat_dummy_sem(nc), 0)
    wait_clock.add_sem_waits(carrier.ins, ScopedClock({None: tick_clock.global_clock}))
    si = carrier.ins.sync_info
    waits = [w for w in si.on_wait if w.ant_name != "compat_dummy_sem"]
    if waits:
        si.on_wait = [waits[0]]
        for w in waits[1:]:
            extra = nc.sync.wait_ge(self._compat_dummy_sem(nc), 0)
            extra.ins.sync_info.on_wait = [w]
    else:
        si.on_wait = []

    nc.all_engine_barrier()
    assert self.sems is not None
    popped = nc._tile_sem_poison_stack.pop()
    assert popped is self._sem_poison
    nc.clear_and_free_semaphores(list(self.sems.allocated().values()))
    nc.all_engine_barrier()


def _split_multi_waits(nc):
    """This walrus build supports at most one sync wait per instruction.
    Tile's wait-assignment can attach several; split the extras into
    standalone EventSemaphore instructions on the same engine, inserted
    immediately before (engines execute their stream in order)."""
    n = [0]
    for f in nc.m.functions:
        for bb in f.blocks:
            out = []
            changed = False
            for inst in bb.instructions:
                si = inst.sync_info
                if si is not None and len(si.on_wait) > 1:
                    waits = list(si.on_wait)
                    for w in waits[:-1]:
                        n[0] += 1
                        ev = mybir.InstEventSemaphore(
                            name=f"EVSPLIT-{n[0]}", ins=[], outs=[]
                        )
                        ev.engine = inst.engine
                        ev.sync_info = bass_rust.SyncInfo(on_wait=[w], on_update=[])
                        nc.register_instruction(ev, overwrite=True)
                        out.append(ev)
                    si.on_wait = [waits[-1]]
                    changed = True
                out.append(inst)
            if changed:
                bb.instructions = out


_install_ntff_hook_shim()
bass.Bass.multi_engine_barrier = _compat_multi_engine_barrier
bass.Bass.all_engine_barrier = _compat_all_engine_barrier
tile.TileContext._drain_and_barrier = _compat_drain_and_barrier
tile.TileContext._compat_dummy_sem = _compat_dummy_sem

# ---------------------------------------------------------------------------
# Problem constants (hardcoded per the grading contract).
# ---------------------------------------------------------------------------

D = 2048          # hidden_d
V = 2048          # hidden_v
L = 512           # seq length
LP1 = L + 1       # 513
T = 65            # decode steps (len(positions) + 1)
NC = 8            # cores
GS = 4 * D // NC  # 1024 gate rows per core
DS = D // NC      # 256 h dims per core
VS = V // NC      # 256 v dims per core
KT = D // 128     # 16 contraction tiles
MT = GS // 128    # 8 gate tiles per core
VT = VS // 128    # 2 v tiles per core

FP32 = mybir.dt.float32
BF16 = mybir.dt.bfloat16
AF = mybir.ActivationFunctionType
ALU = mybir.AluOpType

_cache = {}


def _build_program():
    nc = bass.Bass(num_devices=NC)

    # ---- I/O ----
    inp = {}
    def din(name, shape):
        inp[name] = nc.dram_tensor(name, list(shape), FP32, kind="ExternalInput")
        return inp[name]

    w_sb_in = nc.dram_tensor("w_sb", [128, KT * MT * 128], BF16,
                             kind="ExternalInput")  # W_hh lhsT tiles (kt, m)
    gx_in = din("gx", [128, MT, T])                  # gate_x bias per (p, m, t)
    h0_in = din("h0", [128, KT])                     # initial h chunks
    wet_in = din("wet", [128, KT * VS])              # We_w shard lhsT tiles
    enct_in = din("enct", [128, KT * LP1])           # encoder_states^T tiles
    wdt_hi_in = nc.dram_tensor("wdt_hi", [128, KT * VS], BF16,
                               kind="ExternalInput")  # Wd_w shard hi
    wdt_lo_in = nc.dram_tensor("wdt_lo", [128, KT * VS], BF16,
                               kind="ExternalInput")  # Wd_w shard lo
    vw_in = din("vw", [128, VT])                     # v_w shard
    biasd_in = din("biasd", [128, VT])               # We_b + Wd_b shard
    vb_in = din("vb", [T, 1])                        # v_b broadcast
    tmask_in = din("tmask", [T, LP1])                # one-hot targets
    ident_in = din("ident", [128, 128])              # identity for PE transpose

    y_logits = nc.dram_tensor("y_logits", [T, LP1], FP32, kind="ExternalOutput")
    y_hs = nc.dram_tensor("y_hs", [T, D], FP32, kind="ExternalOutput")
    y_losses = nc.dram_tensor("y_losses", [T, 1], FP32, kind="ExternalOutput")

    rg = [list(range(NC))]

    with tile.TileContext(nc) as tc:
        with (
            tc.tile_pool(name="const", bufs=1) as cpool,
            tc.tile_pool(name="work", bufs=2) as wpool,
            tc.tile_pool(name="tanhp", bufs=3) as tpool,
            tc.tile_pool(name="ps_small", bufs=2, space="PSUM") as ps_small,
            tc.tile_pool(name="ps_enc", bufs=1, space="PSUM") as ps_enc,
            tc.tile_pool(name="ps_lg", bufs=2, space="PSUM") as ps_lg,
            tc.tile_pool(name="dram", bufs=1, space="DRAM") as dram,
        ):
            # ---- resident SBUF tensors ----
            w_sb = cpool.tile([128, KT * MT * 128], BF16)
            gx_sb = cpool.tile([128, MT, T], FP32)
            wet_sb = cpool.tile([128, KT * VS], FP32)
            enct_sb = cpool.tile([128, KT * LP1], FP32)
            wdt_hi_sb = cpool.tile([128, KT * VS], BF16)
            wdt_lo_sb = cpool.tile([128, KT * VS], BF16)
            vw_sb = cpool.tile([128, VT], FP32)
            biasd_sb = cpool.tile([128, VT], FP32)
            vb_sb = cpool.tile([T, 1], FP32)
            tmask_sb = cpool.tile([T, LP1], FP32)
            ident_sb = cpool.tile([128, 128], FP32)
            enc_sb = cpool.tile([128, VT, LP1], FP32)   # enc_proj [v, l]
            ht_sb = cpool.tile([128, KT, T], FP32)      # H^T chunks over time
            d_sb = cpool.tile([128, VT, T], FP32)       # Wd h + bias per (v, t)
            c_sb = cpool.tile([128, 2], FP32)           # cell state shard
            h0_sb = cpool.tile([128, KT], FP32)

            nc.sync.dma_start(w_sb[:], w_sb_in[:])
            nc.sync.dma_start(gx_sb[:], gx_in[:])
            nc.sync.dma_start(wet_sb[:], wet_in[:])
            nc.sync.dma_start(enct_sb[:], enct_in[:])
            nc.sync.dma_start(wdt_hi_sb[:], wdt_hi_in[:])
            nc.sync.dma_start(wdt_lo_sb[:], wdt_lo_in[:])
            nc.sync.dma_start(vw_sb[:], vw_in[:])
            nc.sync.dma_start(biasd_sb[:], biasd_in[:])
            nc.sync.dma_start(vb_sb[:], vb_in[:])
            nc.sync.dma_start(tmask_sb[:], tmask_in[:])
            nc.sync.dma_start(ident_sb[:], ident_in[:])
            nc.sync.dma_start(h0_sb[:], h0_in[:])
            nc.gpsimd.memset(c_sb[:], 0.0)

            # ---- encoder projection: enc_sb[v, l] = (We_w @ enc^T) shard ----
            # (matmul PSUM outputs must stay within one 2KB bank -> chunk l)
            NCHUNKS = ((0, 257), (257, LP1))
            for vt in range(VT):
                for n0, n1 in NCHUNKS:
                    enc_ps = ps_enc.tile([128, 257], FP32, name="enc_ps")
                    for kt in range(KT):
                        nc.tensor.matmul(
                            enc_ps[:, 0 : n1 - n0],
                            wet_sb[:, kt * VS + vt * 128 : kt * VS + vt * 128 + 128],
                            enct_sb[:, kt * LP1 + n0 : kt * LP1 + n1],
                            start=(kt == 0),
                            stop=(kt == KT - 1),
                        )
                    nc.vector.tensor_copy(enc_sb[:, vt, n0:n1], enc_ps[:, 0 : n1 - n0])

            # ---- sequential LSTM with fused per-step pointer work ----
            # Exchange bf16 (hi, lo) h pairs so the gathered data feeds the
            # PE matvec directly; reconstruct fp32 h as hi+lo. Each step's
            # pointer work (d column, 2 tanh tiles, v_w reduction) runs in
            # the exchange/latency shadow on otherwise-idle engines.
            lar_in = dram.tile([T, LP1], FP32, name="lar_in", tag="lar_in")
            lar_out = dram.tile([T, LP1], FP32, name="lar_out", tag="lar_out",
                                addr_space="Shared")

            def emit_pointer_work(t, rhs_t):
                if t < 0:
                    return
                # d column: bf16 hi/lo weight split x (h_hi, h_lo) — 3 terms
                lg_psA = ps_lg.tile([1, 257], FP32, name="lg_psA", bufs=1)
                lg_psB = ps_lg.tile([1, 256], FP32, name="lg_psB", bufs=1)
                for vt in range(VT):
                    d_ps = ps_small.tile([128, 2], FP32, name="d_ps", tag="dcol",
                                         bufs=1)
                    d_ps2 = ps_small.tile([128, 1], FP32, name="d_ps2", tag="dcol2",
                                          bufs=1)
                    for kt in range(KT):
                        w0 = kt * VS + vt * 128
                        nc.tensor.matmul(
                            d_ps[:], wdt_hi_sb[:, w0 : w0 + 128],
                            rhs_t[:, kt, :],
                            start=(kt == 0), stop=(kt == KT - 1),
                        )
                    for kt in range(KT):
                        w0 = kt * VS + vt * 128
                        nc.tensor.matmul(
                            d_ps2[:], wdt_lo_sb[:, w0 : w0 + 128],
                            rhs_t[:, kt, 0:1],
                            start=(kt == 0), stop=(kt == KT - 1),
                        )
                    dsum = wpool.tile([128, 1], FP32, name="dsum")
                    nc.vector.tensor_reduce(dsum[:], d_ps[:],
                                            mybir.AxisListType.X, ALU.add)
                    dsum2 = wpool.tile([128, 1], FP32, name="dsum2")
                    nc.vector.tensor_tensor(dsum2[:], dsum[:], d_ps2[:], ALU.add)
                    nc.vector.tensor_scalar(
                        d_sb[:, vt, t : t + 1], dsum2[:],
                        biasd_sb[:, vt : vt + 1], None, ALU.add,
                    )
                    tanh_t = tpool.tile([128, LP1], FP32, name="tanh_t")
                    nc.scalar.activation(
                        tanh_t[:], enc_sb[:, vt, :], AF.Tanh,
                        bias=d_sb[:, vt, t : t + 1],
                    )
                    for (n0, n1), lg_ps in ((NCHUNKS[0], lg_psA), (NCHUNKS[1], lg_psB)):
                        nc.tensor.matmul(
                            lg_ps[0:1, 0 : n1 - n0],
                            vw_sb[:, vt : vt + 1],
                            tanh_t[:, n0:n1],
                            start=(vt == 0),
                            stop=(vt == VT - 1),
                        )
                lrow = tpool.tile([1, LP1], FP32, name="lrow")
                nc.vector.tensor_copy(lrow[0:1, 0:257], lg_psA[0:1, :])
                nc.vector.tensor_copy(lrow[0:1, 257:LP1], lg_psB[0:1, :])
                nc.sync.dma_start(lar_in[t : t + 1, :], lrow[0:1, :])

            rhs_prev = None
            # t=0 rhs from the initial state
            rhs_cur = wpool.tile([128, KT, 2], BF16, name="rhs_hl", bufs=3)
            nc.vector.tensor_copy(rhs_cur[:, :, 0], h0_sb[:])
            herr0 = wpool.tile([128, KT], FP32, name="herr0")
            nc.vector.tensor_tensor(herr0[:], h0_sb[:], rhs_cur[:, :, 0], ALU.subtract)
            nc.vector.tensor_copy(rhs_cur[:, :, 1], herr0[:])

            for t in range(T):
                # gates matvec: psum[:, 2m:2m+2] = sum_kt W(kt,m)^T @ [h_hi h_lo]
                g_ps = ps_small.tile([128, 2 * MT], FP32, name="g_ps", tag="mm1")
                for m in range(MT):
                    for kt in range(KT):
                        nc.tensor.matmul(
                            g_ps[:, 2 * m : 2 * m + 2],
                            w_sb[:, (kt * MT + m) * 128 : (kt * MT + m) * 128 + 128],
                            rhs_cur[:, kt, :],
                            start=(kt == 0),
                            stop=(kt == KT - 1),
                        )
                gates0 = wpool.tile([128, MT], FP32, name="gates0")
                nc.vector.tensor_reduce(
                    gates0[:], g_ps[:].rearrange("p (m two) -> p m two", two=2),
                    mybir.AxisListType.X, ALU.add,
                )
                gates = wpool.tile([128, MT], FP32, name="gates")
                nc.vector.tensor_tensor(gates[:], gates0[:], gx_sb[:, :, t], ALU.add)
                acts = wpool.tile([128, MT], FP32, name="acts")
                nc.scalar.activation(acts[:, 0:4], gates[:, 0:4], AF.Sigmoid)
                nc.scalar.activation(acts[:, 4:6], gates[:, 4:6], AF.Tanh)
                nc.scalar.activation(acts[:, 6:8], gates[:, 6:8], AF.Sigmoid)
                # c = f*c + i*g ; h = o * tanh(c)
                tmp = wpool.tile([128, 4], FP32, name="tmp")
                nc.vector.tensor_tensor(tmp[:, 0:2], acts[:, 2:4], c_sb[:], ALU.mult)
                nc.vector.tensor_tensor(tmp[:, 2:4], acts[:, 0:2], acts[:, 4:6], ALU.mult)
                nc.vector.tensor_tensor(c_sb[:], tmp[:, 0:2], tmp[:, 2:4], ALU.add)
                tanh_c = wpool.tile([128, 2], FP32, name="tanh_c")
                nc.scalar.activation(tanh_c[:], c_sb[:], AF.Tanh)
                h_sh = wpool.tile([128, 2], FP32, name="h_sh")
                nc.vector.tensor_tensor(h_sh[:], acts[:, 6:8], tanh_c[:], ALU.mult)

                # bf16 hi/lo split of the own shard, then AllGather
                sh_hl = wpool.tile([128, 2, 2], BF16, name="sh_hl", bufs=3)
                nc.vector.tensor_copy(sh_hl[:, :, 0], h_sh[:])
                sherr = wpool.tile([128, 2], FP32, name="sherr")
                nc.vector.tensor_tensor(sherr[:], h_sh[:], sh_hl[:, :, 0], ALU.subtract)
                nc.vector.tensor_copy(sh_hl[:, :, 1], sherr[:])
                ag_in = dram.tile([128, 2, 2], BF16, name=f"agi{t}", tag=f"agi{t}")
                ag_out = dram.tile([NC, 128, 2, 2], BF16, name=f"ago{t}",
                                   tag=f"ago{t}", addr_space="Shared")
                nc.sync.dma_start(ag_in[:], sh_hl[:])
                nc.gpsimd.collective_compute(
                    "AllGather", ALU.bypass, replica_groups=rg,
                    ins=[ag_in[:].opt()], outs=[ag_out[:].opt()],
                )
                rhs_cur = wpool.tile([128, KT, 2], BF16, name="rhs_hl", bufs=3)
                nc.sync.dma_start(
                    rhs_cur[:].rearrange("p (c j) l -> p c j l", j=2),
                    ag_out[:].rearrange("c p j l -> p c j l"),
                )
                # fp32 h for this step = hi + lo (for hs output)
                nc.vector.tensor_tensor(ht_sb[:, :, t], rhs_cur[:, :, 0],
                                        rhs_cur[:, :, 1], ALU.add)

                # pointer work for the PREVIOUS step: its data arrived one
                # exchange ago, so on the in-order PE it fills this step's
                # exchange-wait idle window instead of delaying the matvec.
                emit_pointer_work(t - 1, rhs_prev)
                rhs_prev = rhs_cur
            emit_pointer_work(T - 1, rhs_prev)

            # ---- AllReduce partial logits ----
            nc.gpsimd.collective_compute(
                "AllReduce", ALU.add, replica_groups=rg,
                ins=[lar_in[:].opt()], outs=[lar_out[:].opt()],
            )
            lraw_sb = cpool.tile([T, LP1], FP32)
            nc.sync.dma_start(lraw_sb[:], lar_out[:])
            # + v_b
            lfull_sb = cpool.tile([T, LP1], FP32)
            nc.scalar.activation(lfull_sb[:], lraw_sb[:], AF.Identity, bias=vb_sb[:])
            nc.sync.dma_start(y_logits[:], lfull_sb[:])

            # ---- losses: logsumexp(logits) - logits[target] ----
            mx = wpool.tile([T, 1], FP32, name="mx")
            nc.vector.tensor_reduce(mx[:], lfull_sb[:], mybir.AxisListType.X, ALU.max)
            negm = wpool.tile([T, 1], FP32, name="negm")
            nc.vector.tensor_scalar_mul(negm[:], mx[:], -1.0)
            es = wpool.tile([T, LP1], FP32, name="es")
            sumexp = wpool.tile([T, 1], FP32, name="sumexp")
            nc.scalar.activation(es[:], lfull_sb[:], AF.Exp, bias=negm[:],
                                 accum_out=sumexp[:])
            lse = wpool.tile([T, 1], FP32, name="lse")
            nc.scalar.activation(lse[:], sumexp[:], AF.Ln)
            tl_prod = wpool.tile([T, LP1], FP32, name="tl_prod")
            nc.vector.tensor_tensor(tl_prod[:], lfull_sb[:], tmask_sb[:], ALU.mult)
            tl = wpool.tile([T, 1], FP32, name="tl")
            nc.vector.tensor_reduce(tl[:], tl_prod[:], mybir.AxisListType.X, ALU.add)
            lsum = wpool.tile([T, 1], FP32, name="lsum")
            nc.vector.tensor_tensor(lsum[:], lse[:], mx[:], ALU.add)
            loss = wpool.tile([T, 1], FP32, name="loss")
            nc.vector.tensor_tensor(loss[:], lsum[:], tl[:], ALU.subtract)
            nc.sync.dma_start(y_losses[:], loss[:])

            # ---- hs output: transpose H^T chunks -> [65, 2048] ----
            for kt in range(KT):
                tr_ps = ps_small.tile([T, 128], FP32, name="tr_ps", tag="mm1")
                nc.tensor.transpose(tr_ps[:], ht_sb[:, kt, :], ident_sb[:])
                hs_sb = wpool.tile([T, 128], FP32, name="hs_sb")
                nc.vector.tensor_copy(hs_sb[:], tr_ps[:])
                nc.sync.dma_start(y_hs[:, kt * 128 : (kt + 1) * 128], hs_sb[:])

    _split_multi_waits(nc)
    return nc


def _prep_inputs_core(c, arrs):
    """Build the per-core input map from full numpy inputs."""
    (initial_state, encoder_states, seq_points, positions,
     W_ih, W_hh, b_ih, b_hh, We_w, We_b, Wd_w, Wd_b, v_w, v_b) = arrs

    special = np.array([0.0, 0.0, 1.0], dtype=np.float32)
    stack_seq = np.concatenate([seq_points, special[None]], axis=0)
    xs = np.concatenate([special[None], stack_seq[positions]], axis=0)  # [T, 3]
    targets = np.concatenate([positions.astype(np.int64), [L]], axis=0)

    gate_x = xs @ W_ih.T + b_ih + b_hh                     # [T, 4D]

    # gate-row indices for core c: 256-chunk c of each of i,f,g,o
    idx = np.concatenate([
        np.arange(g * D + c * DS, g * D + (c + 1) * DS) for g in range(4)
    ])
    W_shard = W_hh[idx, :]                                  # [1024, 2048]
    w_sb = (W_shard.reshape(MT, 128, KT, 128)
            .transpose(3, 2, 0, 1).reshape(128, KT * MT * 128))
    gx = gate_x[:, idx].reshape(T, MT, 128).transpose(2, 1, 0)  # [128, MT, T]
    h0 = initial_state.reshape(KT, 128).T                   # [128, KT]

    We_shard = We_w[c * VS:(c + 1) * VS, :]
    wet = (We_shard.T.reshape(KT, 128, VS)
           .transpose(1, 0, 2).reshape(128, KT * VS))
    enct = (encoder_states.T.reshape(KT, 128, LP1)
            .transpose(1, 0, 2).reshape(128, KT * LP1))
    import ml_dtypes
    Wd_shard = Wd_w[c * VS:(c + 1) * VS, :]
    wdt = (Wd_shard.T.reshape(KT, 128, VS)
           .transpose(1, 0, 2).reshape(128, KT * VS))
    wdt_hi = wdt.astype(ml_dtypes.bfloat16)
    wdt_lo = (wdt - wdt_hi.astype(np.float32)).astype(ml_dtypes.bfloat16)
    vw = v_w[0, c * VS:(c + 1) * VS].reshape(VT, 128).T     # [128, VT]
    biasd = (We_b[c * VS:(c + 1) * VS] + Wd_b[c * VS:(c + 1) * VS]
             ).reshape(VT, 128).T
    vb = np.full((T, 1), np.float32(v_b[0]))
    tmask = np.zeros((T, LP1), dtype=np.float32)
    tmask[np.arange(T), targets] = 1.0

    f32c = lambda a: np.ascontiguousarray(a, dtype=np.float32)
    return {
        "w_sb": np.ascontiguousarray(w_sb.astype(ml_dtypes.bfloat16)), "gx": f32c(gx), "h0": f32c(h0),
        "wet": f32c(wet), "enct": f32c(enct),
        "wdt_hi": np.ascontiguousarray(wdt_hi),
        "wdt_lo": np.ascontiguousarray(wdt_lo),
        "vw": f32c(vw), "biasd": f32c(biasd), "vb": f32c(vb),
        "tmask": f32c(tmask), "ident": np.eye(128, dtype=np.float32),
    }


LAST_EXEC_NS = None


def kernel(initial_state, encoder_states, seq_points, positions,
           W_ih, W_hh, b_ih, b_hh, We_w, We_b, Wd_w, Wd_b, v_w, v_b):
    global LAST_EXEC_NS
    arrs = [np.asarray(a) for a in (
        initial_state, encoder_states, seq_points, positions,
        W_ih, W_hh, b_ih, b_hh, We_w, We_b, Wd_w, Wd_b, v_w, v_b)]
    arrs = [a.astype(np.float32) if a.dtype == np.float64 else a for a in arrs]
    positions_np = np.asarray(arrs[3]).astype(np.int64)
    arrs[3] = positions_np

    if "nc" not in _cache:
        _cache["nc"] = _build_program()
    nc = _cache["nc"]

    in_maps = [_prep_inputs_core(c, arrs) for c in range(NC)]
    trace = bool(int(os.environ.get("KERNEL_TRACE", "0")))
    res = run_bass_kernel_spmd(nc, in_maps, core_ids=list(range(NC)), trace=trace)
    LAST_EXEC_NS = res.exec_time_ns
    out = res.results[0]
    logits = out["y_logits"].astype(np.float32)
    hs = out["y_hs"].astype(np.float32)
    losses = out["y_losses"].reshape(T).astype(np.float32)
    return logits, hs, losses
